# revision 1
# baseline (speedup 1.0000x reference)
"""Trainium2 Bass kernel for nn_GAT_7507602833557 (8-core SPMD GAT).

Sharding: query-node rows split across 8 cores (512 rows each); keys/values
replicated. Per-core adjacency slice is passed pre-transposed ([keys, own
queries]) in bf16 ({0,1} values are exact in bf16).

Math notes (per attention map, 9 maps total: 6 spatial + 2 intent + 1 output):
  e[i,j] = leakyrelu(f1[i] + f2[j], 0.2);  softmax over masked j; att @ V.
  exp(leakyrelu(z)) = max(exp(z), exp(0.2 z)) since z >= 0.2z for z>=0 and
  z <= 0.2z for z<0, and exp is monotone. Factorizing,
    exp(lrelu(f1+f2)) = e^{f1[i]} * e^{0.2 f2[j]} * max(e^{0.8 f2[j]}, e^{-0.8 f1[i]})
  and the e^{f1[i]} factor is constant along j so it cancels in the softmax
  normalization. With P=e^{0.8 f2}, C=e^{0.2 f2}, Q=e^{-0.8 f1} the masked
  unnormalized weight is  m[j,i] = adj[j,i] * max(P[j], Q[i]) * C[j],
  one dual-op tensor_scalar (max then mult, both per-partition scalars) plus
  one tensor_tensor mask multiply per tile. Softmax denominator via a ones
  column appended to the value matrix. elu(v) = min(exp(v)-1, max(v, 0)).
"""
import os
import numpy as np

import concourse.bass as bass
import concourse.bacc as bacc
import concourse.tile as tile
from concourse import mybir
from concourse.bass_utils import run_bass_kernel_spmd
from concourse.masks import make_identity

import ml_dtypes

N, NIN, NHID, NOUT = 4096, 64, 32, 64
NHEADS, D_INT = 8, 32
H_SP, H_INT = 6, 2
NCORES = 8
R = N // NCORES           # 512 own query rows per core
JT = N // 128             # 32 key tiles
IT = R // 128             # 4 own query tiles
F32 = mybir.dt.float32
BF16 = mybir.dt.bfloat16
MAPDT = F32 if os.environ.get("KERNEL_F32") else BF16
NPMAP = np.float32 if os.environ.get("KERNEL_F32") else ml_dtypes.bfloat16
# every POOL_STRIDE-th mask-multiply runs on the (otherwise idle) gpsimd
POOL_STRIDE = int(os.environ.get("KERNEL_POOL_STRIDE", "8"))


def _build_program(reps=1):
    nc = bacc.Bacc("TRN2", target_bir_lowering=False, debug=False,
                   num_devices=NCORES)
    d_x = nc.dram_tensor("xT", [NIN, N], F32, kind="ExternalInput")
    d_ie = nc.dram_tensor("ieT", [D_INT, N], F32, kind="ExternalInput")
    d_adjT = nc.dram_tensor("adjT", [N, R], MAPDT, kind="ExternalInput")
    d_xo = nc.dram_tensor("xoT", [NIN, R], F32, kind="ExternalInput")
    d_io = nc.dram_tensor("ioT", [D_INT, R], F32, kind="ExternalInput")
    d_wsp = nc.dram_tensor("wsp", [H_SP, NIN, NHID], F32, kind="ExternalInput")
    d_asp = nc.dram_tensor("asp", [H_SP, 2 * NHID], F32, kind="ExternalInput")
    d_wint = nc.dram_tensor("wint", [H_INT, NIN, NHID], F32, kind="ExternalInput")
    d_aint = nc.dram_tensor("aint", [H_INT, 2 * D_INT], F32, kind="ExternalInput")
    d_wout = nc.dram_tensor("wout", [NHEADS * NHID, NOUT], F32, kind="ExternalInput")
    d_aout = nc.dram_tensor("aout", [2 * NOUT], F32, kind="ExternalInput")
    d_out = nc.dram_tensor("out", [R, NOUT], F32, kind="ExternalOutput")

    with tile.TileContext(nc) as tc:
        for _ in range(reps):
            _kernel_body(tc, d_x, d_ie, d_adjT, d_xo, d_io, d_wsp, d_asp,
                         d_wint, d_aint, d_wout, d_aout, d_out)
    nc.compile()
    return nc


def _kernel_body(tc, d_x, d_ie, d_adjT, d_xo, d_io, d_wsp, d_asp, d_wint,
                 d_aint, d_wout, d_aout, d_out):
    nc = tc.nc
    Act = mybir.ActivationFunctionType
    Alu = mybir.AluOpType

    from contextlib import ExitStack
    ctx = ExitStack()
    big = ctx.enter_context(tc.tile_pool(name="big", bufs=1))
    work = ctx.enter_context(tc.tile_pool(name="work", bufs=int(os.environ.get("KERNEL_WORK_BUFS", "6"))))
    psum = ctx.enter_context(tc.tile_pool(name="psum", bufs=3, space="PSUM"))
    pacc = ctx.enter_context(tc.tile_pool(name="pacc", bufs=1, space="PSUM"))
    ext = ctx.enter_context(tc.tile_pool(name="ext", bufs=2))
    dram = ctx.enter_context(tc.tile_pool(name="dram", bufs=1, space="DRAM"))
    try:
        _body(tc, ctx, big, work, psum, pacc, dram, d_x, d_ie, d_adjT, d_xo,
              d_io, d_wsp, d_asp, d_wint, d_aint, d_wout, d_aout, d_out, ext)
    finally:
        ctx.close()


def _body(tc, ctx, big, work, psum, pacc, dram, d_x, d_ie, d_adjT, d_xo, d_io,
          d_wsp, d_asp, d_wint, d_aint, d_wout, d_aout, d_out, ext):
    nc = tc.nc
    Act = mybir.ActivationFunctionType
    Alu = mybir.AluOpType

    # ---------------- loads (critical-path-first order) --------------------
    # small weight/vector tensors first: they gate the wtilde/ET/Q chains
    aintp = big.tile([D_INT, 2 * H_INT], F32, tag="aintp")
    nc.sync.dma_start(out=aintp, in_=d_aint.ap().rearrange("h (c o) -> o (h c)", c=2))
    apair = big.tile([NHID, 2 * H_SP], F32, tag="apair")
    nc.sync.dma_start(out=apair, in_=d_asp.ap().rearrange("h (c o) -> o (h c)", c=2))
    ioT = big.tile([D_INT, R], F32, tag="ioT")
    nc.sync.dma_start(out=ioT, in_=d_io.ap())
    xoT = big.tile([NIN, R], F32, tag="xoT")
    nc.sync.dma_start(out=xoT, in_=d_xo.ap())
    adjT_sb = big.tile([128, JT, R], MAPDT, tag="adjT_sb")

    def load_adj(g):
        nc.sync.dma_start(
            out=adjT_sb[:, 4 * g:4 * (g + 1), :],
            in_=d_adjT.ap()[4 * g * 128:4 * (g + 1) * 128, :]
                .rearrange("(t p) i -> p t i", p=128))
    load_adj(0)
    w_all3 = big.tile([NIN, NHEADS, NHID], F32, tag="w_all3")
    nc.sync.dma_start(out=w_all3[:, 0:H_SP, :],
                      in_=d_wsp.ap().rearrange("h f o -> f h o"))
    nc.sync.dma_start(out=w_all3[:, H_SP:, :],
                      in_=d_wint.ap().rearrange("h f o -> f h o"))
    w_all = w_all3.rearrange("f h o -> f (h o)")
    ieT = big.tile([D_INT, N], F32, tag="ieT")
    nc.sync.dma_start(out=ieT, in_=d_ie.ap())
    xT = big.tile([NIN, N], F32, tag="xT")
    for g in range(2):
        nc.sync.dma_start(out=xT[:, 2048 * g:2048 * (g + 1)],
                          in_=d_x.ap()[:, 2048 * g:2048 * (g + 1)])
    for g in range(1, 8):
        load_adj(g)
    wout_f = big.tile([128, 2, NOUT], F32, tag="wout_f")
    nc.sync.dma_start(out=wout_f, in_=d_wout.ap().rearrange("(c p) o -> p c o", p=128))
    aout_sb = big.tile([NOUT, 2], F32, tag="aout_sb")
    nc.sync.dma_start(out=aout_sb, in_=d_aout.ap().rearrange("(c o) -> o c", c=2))

    ident = big.tile([128, 128], F32, tag="ident")
    make_identity(nc, ident)

    def tr(out, in_, idt):
        p = in_.partition_size()
        nc.tensor.transpose(out, in_, idt[0:p, 0:p])

    # ---------------- wtilde: spatial a-vectors pre-projected through W ----
    # W_hT [32, 6, 64]
    wt = big.tile([NHID, H_SP, NIN], F32, tag="wt")
    for grp in range(2):
        ptw = psum.tile([NHID, 3 * NIN], F32, tag="ps")
        for k in range(3):
            h = 3 * grp + k
            tr(ptw[:, NIN * k:NIN * (k + 1)],
               w_all[:, NHID * h:NHID * (h + 1)], ident)
        nc.scalar.copy(out=wt[:, 3 * grp:3 * grp + 3, :], in_=ptw)
    # psum_w [64, 12]: cols (2h, 2h+1) = (W_h@a1_h, W_h@a2_h)
    pw = psum.tile([NIN, 2 * H_SP], F32, tag="ps")
    for h in range(H_SP):
        nc.tensor.matmul(pw[:, 2 * h:2 * h + 2], wt[:, h, :],
                         apair[:, 2 * h:2 * h + 2])
    # wtilde [64, 12]: 0:6 = 0.8*w2 (P), 6:12 = 0.2*w2 (C); wq [64, 6] = -0.8*w1
    wtilde = big.tile([NIN, 2 * H_SP], F32, tag="wtilde")
    wq = big.tile([NIN, H_SP], F32, tag="wq")
    pw_hc = pw.rearrange("f (h c) -> f c h", c=2)
    w1cols = pw_hc[:, 0, :]
    w2cols = pw_hc[:, 1, :]
    nc.scalar.mul(out=wtilde[:, 0:H_SP], in_=w2cols, mul=0.8)
    nc.scalar.mul(out=wtilde[:, H_SP:], in_=w2cols, mul=0.2)
    nc.scalar.mul(out=wq, in_=w1cols, mul=-0.8)
    # intent: aint_arr [32, 4]: 0:2 = 0.8*a2 (P), 2:4 = 0.2*a2 (C); aq [32,2]
    aint_arr = big.tile([D_INT, 2 * H_INT], F32, tag="aint_arr")
    aq = big.tile([D_INT, H_INT], F32, tag="aq")
    ai_hc = aintp[:].rearrange("f (h c) -> f c h", c=2)
    nc.scalar.mul(out=aint_arr[:, 0:H_INT], in_=ai_hc[:, 1, :], mul=0.8)
    nc.scalar.mul(out=aint_arr[:, H_INT:], in_=ai_hc[:, 1, :], mul=0.2)
    nc.scalar.mul(out=aq, in_=ai_hc[:, 0, :], mul=-0.8)

    # ---------------- Q rows + broadcast -----------------------------------
    pq = psum.tile([H_SP, R], F32, tag="ps")
    nc.tensor.matmul(pq, wq, xoT)
    pqi = psum.tile([H_INT, R], F32, tag="ps")
    nc.tensor.matmul(pqi, aq, ioT)
    qrow_sp = big.tile([H_SP, R], MAPDT, tag="qrow_sp")
    nc.scalar.activation(out=qrow_sp, in_=pq, func=Act.Exp)
    qrow_in = big.tile([H_INT, R], MAPDT, tag="qrow_in")
    nc.scalar.activation(out=qrow_in, in_=pqi, func=Act.Exp)
    ones1 = big.tile([1, 128], F32, tag="ones1")
    nc.vector.memset(ones1, 1.0)
    qb = big.tile([128, NHEADS, R], MAPDT, tag="qb")
    qrd_in = dram.tile([H_INT, R], MAPDT, tag="qrd_in")
    nc.sync.dma_start(out=qrd_in, in_=qrow_in)
    qrd_sp = dram.tile([H_SP, R], MAPDT, tag="qrd_sp")
    nc.sync.dma_start(out=qrd_sp, in_=qrow_sp)
    for h in [6, 7, 0, 1, 2, 3, 4, 5]:
        src = qrd_in[h - H_SP:h - H_SP + 1, :] if h >= H_SP \
            else qrd_sp[h:h + 1, :]
        nc.gpsimd.dma_start(out=qb[:, h, :], in_=src.to_broadcast([128, R]))

    # ---------------- ET [128, 32, 16]: per-key P/C columns ---------------
    # cols 0-5 P_sp, 6-11 C_sp, 12-13 P_int, 14-15 C_int
    et = big.tile([128, JT, 16], F32, tag="et")
    for jt in range(JT):
        pf = psum.tile([128, 16], F32, tag="ps")
        nc.tensor.matmul(pf[:, 0:12], xT[:, 128 * jt:128 * (jt + 1)], wtilde)
        nc.tensor.matmul(pf[:, 12:16], ieT[:, 128 * jt:128 * (jt + 1)], aint_arr)
        nc.scalar.activation(out=et[:, jt, 12:16], in_=pf[:, 12:16], func=Act.Exp)
        nc.scalar.activation(out=et[:, jt, 0:12], in_=pf[:, 0:12], func=Act.Exp)

    # ---------------- Whplus [128, 32, 8, 33] ------------------------------
    whp = big.tile([128, JT, NHEADS, NHID + 1], MAPDT, tag="whp")
    nc.vector.memset(whp, 1.0)
    for jt in range(JT):
        pwh = psum.tile([128, NHEADS * NHID], F32, tag="ps")
        nc.tensor.matmul(pwh, xT[:, 128 * jt:128 * (jt + 1)], w_all)
        nc.scalar.copy(out=whp[:, jt, :, 0:NHID],
                       in_=pwh.rearrange("p (h o) -> p h o", h=NHEADS))

    # ---------------- layer 1 attention ------------------------------------
    accs = [pacc.tile([128, NHEADS, NHID + 1], F32, tag=f"acc{i}",
                      name=f"acc_l1_{i}") for i in range(IT)]
    # intent heads first: their ET/Q inputs have the shortest dependency chain
    for h in [6, 7, 0, 1, 2, 3, 4, 5]:
        pcol = h if h < H_SP else 12 + (h - H_SP)
        ccol = (H_SP + h) if h < H_SP else 14 + (h - H_SP)
        for jc in range(JT):
            t = work.tile([128, R], MAPDT, tag="t")
            nc.vector.tensor_scalar(
                out=t, in0=qb[:, h, :],
                scalar1=et[:, jc, pcol:pcol + 1],
                scalar2=et[:, jc, ccol:ccol + 1],
                op0=Alu.max, op1=Alu.mult)
            m = work.tile([128, R], MAPDT, tag="m")
            eng = (nc.gpsimd
                   if POOL_STRIDE and (h * JT + jc) % POOL_STRIDE == 0
                   else nc.vector)
            eng.tensor_tensor(m, t, adjT_sb[:, jc, :], Alu.mult)
            for it in range(IT):
                nc.tensor.matmul(accs[it][:, h, :],
                                 m[:, 128 * it:128 * (it + 1)],
                                 whp[:, jc, h, :],
                                 start=(jc == 0), stop=(jc == JT - 1))

    # ---------------- h = elu(num/den) -------------------------------------
    hT = big.tile([128, 2, R], MAPDT, tag="hT")
    h_nat = big.tile([128, IT, NHEADS * NHID], MAPDT, tag="h_nat")
    for it in range(IT):
        rec = ext.tile([128, NHEADS], F32, tag="rec")
        nc.vector.reciprocal(out=rec, in_=accs[it][:, :, NHID])
        v = ext.tile([128, NHEADS * NHID], F32, tag="v")
        vv = v.rearrange("p (h o) -> p h o", h=NHEADS)
        for h in range(NHEADS):
            nc.scalar.mul(out=vv[:, h, :], in_=accs[it][:, h, 0:NHID],
                          mul=rec[:, h:h + 1])
        e = ext.tile([128, NHEADS * NHID], F32, tag="e")
        nc.scalar.activation(out=e, in_=v, func=Act.Exp)
        em1 = ext.tile([128, NHEADS * NHID], F32, tag="em1")
        nc.vector.tensor_scalar(out=em1, in0=e, scalar1=-1.0, scalar2=None,
                                op0=Alu.add)
        r = ext.tile([128, NHEADS * NHID], F32, tag="r")
        nc.vector.tensor_scalar(out=r, in0=v, scalar1=0.0, scalar2=None,
                                op0=Alu.max)
        nc.vector.tensor_tensor(h_nat[:, it, :], em1, r, Alu.min)

    # ---------------- Who, o1/o2 -------------------------------------------
    if MAPDT == F32:
        id_map = ident
    else:
        ident_b = big.tile([128, 128], BF16, tag="ident_b")
        make_identity(nc, ident_b)
        id_map = ident_b
    for fc in range(2):
        ph = psum.tile([128, R], MAPDT, tag="ps")
        for it in range(IT):
            tr(ph[:, 128 * it:128 * (it + 1)],
               h_nat[:, it, 128 * fc:128 * (fc + 1)], id_map)
        nc.scalar.copy(out=hT[:, fc, :], in_=ph)
    wout_m = big.tile([128, 2, NOUT], MAPDT, tag="wout_m")
    nc.scalar.copy(out=wout_m, in_=wout_f)
    pwho = psum.tile([NOUT, R], F32, tag="ps")
    for fc in range(2):
        nc.tensor.matmul(pwho, wout_m[:, fc, :], hT[:, fc, :],
                         start=(fc == 0), stop=(fc == 1))
    whoT = big.tile([NOUT, R], MAPDT, tag="whoT")
    nc.scalar.copy(out=whoT, in_=pwho)
    whoT_f = big.tile([NOUT, R], F32, tag="whoT_f")
    nc.scalar.copy(out=whoT_f, in_=pwho)
    po1 = psum.tile([1, R], F32, tag="ps")
    nc.tensor.matmul(po1, aout_sb[:, 0:1], whoT_f)
    po2s = big.tile([1, R], F32, tag="po2s")
    po2 = psum.tile([1, R], F32, tag="ps")
    nc.tensor.matmul(po2, aout_sb[:, 1:2], whoT_f)
    nc.scalar.copy(out=po2s, in_=po2)
    # Qo row = exp(-0.8 o1), broadcast via K=1 matmul
    qo_row = big.tile([1, R], F32, tag="qo_row")
    nc.scalar.activation(out=qo_row, in_=po1, func=Act.Exp, scale=-0.8)
    qob = big.tile([128, R], MAPDT, tag="qob")
    pqob = psum.tile([128, R], F32, tag="ps")
    nc.tensor.matmul(pqob, ones1, qo_row)
    nc.scalar.copy(out=qob, in_=pqob)

    # ---------------- payload [R, 67] built transposed ---------------------
    # cols 0:64 Who, 64 ones, 65 Po = exp(0.8 o2), 66 Co = exp(0.2 o2)
    payT = big.tile([128, IT, NOUT + 3], MAPDT, tag="payT")
    for k in range(IT):
        ppt = psum.tile([128, NOUT], MAPDT, tag="ps")
        tr(ppt, whoT[:, 128 * k:128 * (k + 1)], id_map)
        po2t = psum.tile([128, 1], F32, tag="ps")
        tr(po2t, po2s[:, 128 * k:128 * (k + 1)], ident)
        nc.scalar.copy(out=payT[:, k, 0:NOUT], in_=ppt)
        nc.vector.memset(payT[:, k, NOUT:NOUT + 1], 1.0)
        nc.scalar.activation(out=payT[:, k, NOUT + 1:NOUT + 2], in_=po2t,
                             func=Act.Exp, scale=0.8)
        nc.scalar.activation(out=payT[:, k, NOUT + 2:NOUT + 3], in_=po2t,
                             func=Act.Exp, scale=0.2)
    ccin = dram.tile([R, NOUT + 3], MAPDT, tag="ccin")
    ccout = dram.tile([N, NOUT + 3], MAPDT, tag="ccout")
    nc.sync.dma_start(out=ccin.rearrange("(k p) c -> p k c", p=128), in_=payT)
    if os.environ.get("KERNEL_SIMCC"):
        # timeline-sim stand-in: copy local block in place of the collective
        for d in range(NCORES):
            nc.sync.dma_start(out=ccout[R * d:R * (d + 1), :], in_=ccin)
    else:
        nc.gpsimd.collective_compute(
            "AllGather", mybir.AluOpType.bypass,
            replica_groups=[list(range(NCORES))],
            ins=[ccin.opt()], outs=[ccout.opt()])
    whop = big.tile([128, JT, NOUT + 3], MAPDT, tag="whop")
    nc.sync.dma_start(out=whop, in_=ccout.rearrange("(t p) c -> p t c", p=128))
    # per-key P/C as f32 scalars
    pco = big.tile([128, JT, 2], F32, tag="pco")
    nc.scalar.copy(out=pco, in_=whop[:, :, NOUT + 1:NOUT + 3])

    # ---------------- output attention -------------------------------------
    acc2 = [pacc.tile([128, NOUT + 1], F32, tag=f"acc{i}",
                      name=f"acc_l2_{i}") for i in range(IT)]
    for jc in range(JT):
        t = work.tile([128, R], MAPDT, tag="t")
        nc.vector.tensor_scalar(
            out=t, in0=qob,
            scalar1=pco[:, jc, 0:1], scalar2=pco[:, jc, 1:2],
            op0=Alu.max, op1=Alu.mult)
        m = work.tile([128, R], MAPDT, tag="m")
        eng = (nc.gpsimd if POOL_STRIDE and jc % POOL_STRIDE == 0
               else nc.vector)
        eng.tensor_tensor(m, t, adjT_sb[:, jc, :], Alu.mult)
        for it in range(IT):
            nc.tensor.matmul(acc2[it],
                             m[:, 128 * it:128 * (it + 1)],
                             whop[:, jc, 0:NOUT + 1],
                             start=(jc == 0), stop=(jc == JT - 1))

    # ---------------- out = tanh(num/den) ----------------------------------
    out_sb = big.tile([128, IT, NOUT], F32, tag="out_sb")
    for it in range(IT):
        rec2 = ext.tile([128, 1], F32, tag="rec2")
        nc.vector.reciprocal(out=rec2, in_=acc2[it][:, NOUT:NOUT + 1])
        nc.scalar.activation(out=out_sb[:, it, :], in_=acc2[it][:, 0:NOUT],
                             func=Act.Tanh, scale=rec2)
    nc.sync.dma_start(out=d_out.ap().rearrange("(k p) c -> p k c", p=128),
                      in_=out_sb)


_NC_CACHE = None


def _get_nc():
    global _NC_CACHE
    if _NC_CACHE is None:
        _NC_CACHE = _build_program()
    return _NC_CACHE


def _make_in_maps(inputs):
    x = np.asarray(inputs["x"], np.float32)
    adj = np.asarray(inputs["adj"], np.float32)
    ie = np.asarray(inputs["intent_embeds"], np.float32)
    xT_full = np.ascontiguousarray(x.T)
    ieT_full = np.ascontiguousarray(ie.T)
    in_maps = []
    for d in range(NCORES):
        sl = slice(d * R, (d + 1) * R)
        in_maps.append({
            "xT": xT_full, "ieT": ieT_full,
            "adjT": np.ascontiguousarray(adj[sl, :].T).astype(NPMAP),
            "xoT": np.ascontiguousarray(x[sl].T),
            "ioT": np.ascontiguousarray(ie[sl].T),
            "wsp": np.asarray(inputs["W_sp"], np.float32),
            "asp": np.asarray(inputs["a_sp"], np.float32),
            "wint": np.asarray(inputs["W_int"], np.float32),
            "aint": np.asarray(inputs["a_int"], np.float32),
            "wout": np.asarray(inputs["W_out"], np.float32),
            "aout": np.asarray(inputs["a_out"], np.float32),
        })
    return in_maps


def kernel(x, adj, intent_embeds, W_sp, a_sp, W_int, a_int, W_out, a_out):
    nc = _get_nc()
    in_maps = _make_in_maps(dict(
        x=x, adj=adj, intent_embeds=intent_embeds, W_sp=W_sp, a_sp=a_sp,
        W_int=W_int, a_int=a_int, W_out=W_out, a_out=a_out))
    res = run_bass_kernel_spmd(nc, in_maps, list(range(NCORES)))
    return np.concatenate([res.results[d]["out"] for d in range(NCORES)], axis=0)



# revision 31
# speedup vs baseline: 4082.9148x; 4082.9148x over previous
"""Trainium2 Bass kernel for nn_GAT_7507602833557 (8-core SPMD GAT).

Sharding: query-node rows split across 8 cores (512 rows each); keys/values
replicated. Per-core adjacency slice is passed pre-transposed ([keys, own
queries]) in bf16 ({0,1} values are exact in bf16).

Math notes (per attention map, 9 maps total: 6 spatial + 2 intent + 1 output):
  e[i,j] = leakyrelu(f1[i] + f2[j], 0.2);  softmax over masked j; att @ V.
  exp(leakyrelu(z)) = max(exp(z), exp(0.2 z)) since z >= 0.2z for z>=0 and
  z <= 0.2z for z<0, and exp is monotone. Factorizing,
    exp(lrelu(f1+f2)) = e^{f1[i]} * e^{0.2 f2[j]} * max(e^{0.8 f2[j]}, e^{-0.8 f1[i]})
  and the e^{f1[i]} factor is constant along j so it cancels in the softmax
  normalization. With P=e^{0.8 f2}, C=e^{0.2 f2}, Q=e^{-0.8 f1} the masked
  unnormalized weight is  m[j,i] = adj[j,i] * max(P[j], Q[i]) * C[j],
  one dual-op tensor_scalar (max then mult, both per-partition scalars) plus
  one tensor_tensor mask multiply per tile. Softmax denominator via a ones
  column appended to the value matrix. elu(v) = min(exp(v)-1, max(v, 0)).

Schedule: the key dimension is processed in strips of 4 key-tiles. Each
strip computes its ET (P/C columns) and Whplus values just-in-time, then
runs the 8 heads' map ops for the strip, so the vector engine starts ~5us
into the kernel instead of waiting for the full setup. The mask multiply is
batched over the strip's 4 contiguous key tiles (one [128, 2048] op). A
tunable share of the tensor_scalar ops runs on the otherwise-idle gpsimd.
"""
import os
import numpy as np

import concourse.bass as bass
import concourse.bacc as bacc
import concourse.tile as tile
from concourse import mybir
from concourse.bass_utils import run_bass_kernel_spmd
from concourse.masks import make_identity

import ml_dtypes

N, NIN, NHID, NOUT = 4096, 64, 32, 64
NHEADS, D_INT = 8, 32
H_SP, H_INT = 6, 2
NCORES = 8
R = N // NCORES           # 512 own query rows per core
JT = N // 128             # 32 key tiles
IT = R // 128             # 4 own query tiles
STRIP = 4                 # key tiles per strip
NSTRIP = JT // STRIP
F32 = mybir.dt.float32
BF16 = mybir.dt.bfloat16
MAPDT = F32 if os.environ.get("KERNEL_F32") else BF16
NPMAP = np.float32 if os.environ.get("KERNEL_F32") else ml_dtypes.bfloat16
# per strip, this many of the 8 heads run their mask-multiply on gpsimd
# (as 4 single-tile tensor_tensors; the rest batch on DVE). gpsimd
# tensor_scalar is avoided entirely: its dual-op ucode is ~8us per tile
# and stalls the DVE via the shared SBUF ports.
GP_HEADS = int(os.environ.get("KERNEL_GP_HEADS", "0"))
# out-layer: of the 4 key tiles per strip, this many mask-mults go to gpsimd
GP_OUT = int(os.environ.get("KERNEL_GP_OUT", "0"))


def _build_program(reps=1):
    nc = bacc.Bacc("TRN2", target_bir_lowering=False, debug=False,
                   num_devices=NCORES)
    d_x = nc.dram_tensor("xT", [NIN, N], F32, kind="ExternalInput")
    d_ie = nc.dram_tensor("ieT", [D_INT, N], F32, kind="ExternalInput")
    d_adjT = nc.dram_tensor("adjT", [N, R], MAPDT, kind="ExternalInput")
    d_xo = nc.dram_tensor("xoT", [NIN, R], F32, kind="ExternalInput")
    d_io = nc.dram_tensor("ioT", [D_INT, R], F32, kind="ExternalInput")
    d_wsp = nc.dram_tensor("wsp", [H_SP, NIN, NHID], F32, kind="ExternalInput")
    d_asp = nc.dram_tensor("asp", [H_SP, 2 * NHID], F32, kind="ExternalInput")
    d_wint = nc.dram_tensor("wint", [H_INT, NIN, NHID], F32, kind="ExternalInput")
    d_aint = nc.dram_tensor("aint", [H_INT, 2 * D_INT], F32, kind="ExternalInput")
    d_wout = nc.dram_tensor("wout", [NHEADS * NHID, NOUT], F32, kind="ExternalInput")
    d_aout = nc.dram_tensor("aout", [2 * NOUT], F32, kind="ExternalInput")
    d_out = nc.dram_tensor("out", [R, NOUT], F32, kind="ExternalOutput")
    if os.environ.get("KERNEL_DEBUG"):
        nc.dbg_et = nc.dram_tensor("dbg_et", [128, JT, 16], F32, kind="ExternalOutput")
        nc.dbg_qb = nc.dram_tensor("dbg_qb", [128, NHEADS, R], F32, kind="ExternalOutput")
        nc.dbg_acc = nc.dram_tensor("dbg_acc", [128, IT, NHEADS * (NHID + 1)], F32, kind="ExternalOutput")
    else:
        nc.dbg_et = nc.dbg_qb = nc.dbg_acc = None

    with tile.TileContext(nc) as tc:
        for _ in range(reps):
            _kernel_body(tc, d_x, d_ie, d_adjT, d_xo, d_io, d_wsp, d_asp,
                         d_wint, d_aint, d_wout, d_aout, d_out)
    nc.compile()
    return nc


def _kernel_body(tc, d_x, d_ie, d_adjT, d_xo, d_io, d_wsp, d_asp, d_wint,
                 d_aint, d_wout, d_aout, d_out):
    nc = tc.nc
    from contextlib import ExitStack
    ctx = ExitStack()
    big = ctx.enter_context(tc.tile_pool(name="big", bufs=1))
    work = ctx.enter_context(tc.tile_pool(name="work", bufs=int(os.environ.get("KERNEL_WORK_BUFS", "4"))))
    mpool = ctx.enter_context(tc.tile_pool(name="mpool", bufs=int(os.environ.get("KERNEL_M_BUFS", "12"))))
    psum = ctx.enter_context(tc.tile_pool(name="psum", bufs=2, space="PSUM"))
    pet = ctx.enter_context(tc.tile_pool(name="pet", bufs=1, space="PSUM"))
    pacc = ctx.enter_context(tc.tile_pool(name="pacc", bufs=1, space="PSUM"))
    ext = ctx.enter_context(tc.tile_pool(name="ext", bufs=2))
    dram = ctx.enter_context(tc.tile_pool(name="dram", bufs=1, space="DRAM"))
    try:
        _body(tc, ctx, big, work, psum, pacc, dram, d_x, d_ie, d_adjT, d_xo,
              d_io, d_wsp, d_asp, d_wint, d_aint, d_wout, d_aout, d_out, ext,
              mpool, pet)
    finally:
        ctx.close()


def _body(tc, ctx, big, work, psum, pacc, dram, d_x, d_ie, d_adjT, d_xo, d_io,
          d_wsp, d_asp, d_wint, d_aint, d_wout, d_aout, d_out, ext, mpool, pet):
    nc = tc.nc
    Act = mybir.ActivationFunctionType
    Alu = mybir.AluOpType

    # ---------------- loads (critical-path-first order) --------------------
    aintp = big.tile([D_INT, 2 * H_INT], F32, tag="aintp")
    nc.sync.dma_start(out=aintp, in_=d_aint.ap().rearrange("h (c o) -> o (h c)", c=2))
    ioT = big.tile([D_INT, R], F32, tag="ioT")
    nc.sync.dma_start(out=ioT, in_=d_io.ap())
    apair = big.tile([NHID, 2 * H_SP], F32, tag="apair")
    nc.sync.dma_start(out=apair, in_=d_asp.ap().rearrange("h (c o) -> o (h c)", c=2))
    xoT = big.tile([NIN, R], F32, tag="xoT")
    nc.sync.dma_start(out=xoT, in_=d_xo.ap())
    adjT_sb = big.tile([128, JT, R], MAPDT, tag="adjT_sb")

    def load_adj(g):
        nc.gpsimd.dma_start(
            out=adjT_sb[:, 4 * g:4 * (g + 1), :],
            in_=d_adjT.ap()[4 * g * 128:4 * (g + 1) * 128, :]
                .rearrange("(t p) i -> p t i", p=128))
    load_adj(0)
    w_all3 = big.tile([NIN, NHEADS, NHID], F32, tag="w_all3")
    nc.sync.dma_start(out=w_all3[:, 0:H_SP, :],
                      in_=d_wsp.ap().rearrange("h f o -> f h o"))
    nc.sync.dma_start(out=w_all3[:, H_SP:, :],
                      in_=d_wint.ap().rearrange("h f o -> f h o"))
    w_all = w_all3.rearrange("f h o -> f (h o)")
    ieT = big.tile([D_INT, N], F32, tag="ieT")
    nc.gpsimd.dma_start(out=ieT, in_=d_ie.ap())
    xT = big.tile([NIN, N], F32, tag="xT")
    for g in range(2):
        nc.gpsimd.dma_start(out=xT[:, 2048 * g:2048 * (g + 1)],
                            in_=d_x.ap()[:, 2048 * g:2048 * (g + 1)])
    for g in range(1, 8):
        load_adj(g)
    wout_f = big.tile([128, 2, NOUT], F32, tag="wout_f")
    nc.gpsimd.dma_start(out=wout_f, in_=d_wout.ap().rearrange("(c p) o -> p c o", p=128))
    aout_sb = big.tile([NOUT, 2], F32, tag="aout_sb")
    nc.gpsimd.dma_start(out=aout_sb, in_=d_aout.ap().rearrange("(c o) -> o c", c=2))

    # ---------------- intent-head fast path (gates first DVE work) --------
    # aint_arr [32, 4]: 0:2 = 0.8*a2 (P), 2:4 = 0.2*a2 (C); aq [32,2] = -0.8*a1
    aint_arr = big.tile([D_INT, 2 * H_INT], F32, tag="aint_arr")
    aq = big.tile([D_INT, H_INT], F32, tag="aq")
    ai_hc = aintp[:].rearrange("f (h c) -> f c h", c=2)
    nc.scalar.mul(out=aint_arr[:, 0:H_INT], in_=ai_hc[:, 1, :], mul=0.8)
    nc.scalar.mul(out=aint_arr[:, H_INT:], in_=ai_hc[:, 1, :], mul=0.2)
    nc.scalar.mul(out=aq, in_=ai_hc[:, 0, :], mul=-0.8)
    pqi = psum.tile([H_INT, R], F32, tag="ps")
    nc.tensor.matmul(pqi, aq, ioT)
    qrow_in = big.tile([H_INT, R], MAPDT, tag="qrow_in")
    nc.scalar.activation(out=qrow_in, in_=pqi, func=Act.Exp)
    qb = big.tile([128, NHEADS, R], MAPDT, tag="qb")
    ones1 = big.tile([1, 128], MAPDT, tag="ones1")
    nc.vector.memset(ones1, 1.0)
    qrd_in = dram.tile([H_INT, R], MAPDT, tag="qrd_in")
    nc.sync.dma_start(out=qrd_in, in_=qrow_in)
    HEADS = [6, 7, 0, 1, 2, 3, 4, 5]   # intent heads first (shortest dep chain)
    for h in (6, 7):
        nc.sync.dma_start(out=qb[:, h, :],
                          in_=qrd_in[h - H_SP:h - H_SP + 1, :].to_broadcast([128, R]))
    # intent ET per strip: separate tiles keep each strip's ts dependency
    # pinned to exactly its own exp (tile-granular dep tracking otherwise
    # serializes the first ts behind ALL exps)
    et_int = [big.tile([128, STRIP, 2 * H_INT], F32, tag=f"eti{s}",
                       name=f"et_int_{s}") for s in range(NSTRIP)]
    et_sp = [big.tile([128, STRIP, 2 * H_SP], F32, tag=f"etsp{s}",
                      name=f"et_sp_{s}") for s in range(NSTRIP)]
    pint = pet.tile([128, JT, 2 * H_INT], F32, tag="pint")
    for s in range(NSTRIP):
        for jt in range(STRIP * s, STRIP * (s + 1)):
            nc.tensor.matmul(pint[:, jt, :], ieT[:, 128 * jt:128 * (jt + 1)], aint_arr)
        nc.scalar.activation(out=et_int[s], in_=pint[:, STRIP * s:STRIP * (s + 1), :],
                             func=Act.Exp)

    # ---------------- wtilde: spatial a-vectors pre-projected through W ----
    ident = big.tile([128, 128], F32, tag="ident")
    make_identity(nc, ident)

    def tr(out, in_, idt):
        p = in_.partition_size()
        nc.tensor.transpose(out, in_, idt[0:p, 0:p])

    wt = big.tile([NHID, H_SP, NIN], F32, tag="wt")
    for grp in range(2):
        ptw = psum.tile([NHID, 3 * NIN], F32, tag="ps")
        for k in range(3):
            h = 3 * grp + k
            tr(ptw[:, NIN * k:NIN * (k + 1)],
               w_all[:, NHID * h:NHID * (h + 1)], ident)
        nc.scalar.copy(out=wt[:, 3 * grp:3 * grp + 3, :], in_=ptw)
    pw = psum.tile([NIN, 2 * H_SP], F32, tag="ps")
    for h in range(H_SP):
        nc.tensor.matmul(pw[:, 2 * h:2 * h + 2], wt[:, h, :],
                         apair[:, 2 * h:2 * h + 2])
    # wtilde [64, 12]: 0:6 = 0.8*w2 (P), 6:12 = 0.2*w2 (C); wq [64, 6] = -0.8*w1
    wtilde = big.tile([NIN, 2 * H_SP], F32, tag="wtilde")
    wq = big.tile([NIN, H_SP], F32, tag="wq")
    pw_hc = pw.rearrange("f (h c) -> f c h", c=2)
    w1cols = pw_hc[:, 0, :]
    w2cols = pw_hc[:, 1, :]
    nc.scalar.mul(out=wtilde[:, 0:H_SP], in_=w2cols, mul=0.8)
    nc.scalar.mul(out=wtilde[:, H_SP:], in_=w2cols, mul=0.2)
    nc.scalar.mul(out=wq, in_=w1cols, mul=-0.8)

    # ---------------- spatial Q rows + broadcast ---------------------------
    pq = psum.tile([H_SP, R], F32, tag="ps")
    nc.tensor.matmul(pq, wq, xoT)
    qrow_sp = big.tile([H_SP, R], MAPDT, tag="qrow_sp")
    nc.scalar.activation(out=qrow_sp, in_=pq, func=Act.Exp)
    qrd_sp = dram.tile([H_SP, R], MAPDT, tag="qrd_sp")
    nc.sync.dma_start(out=qrd_sp, in_=qrow_sp)
    for h in range(H_SP):
        nc.sync.dma_start(out=qb[:, h, :],
                          in_=qrd_sp[h:h + 1, :].to_broadcast([128, R]))
    # spatial ET for all key tiles

    # ---------------- l1: Whplus + attention ------------------------------
    # et cols 0-5 P_sp, 6-11 C_sp, 12-13 P_int, 14-15 C_int
    whp = big.tile([128, JT, NHEADS, NHID + 1], MAPDT, tag="whp")
    nc.vector.memset(whp[:, :, :, NHID:NHID + 1], 1.0)
    accs = [pacc.tile([128, NHEADS, NHID + 1], F32, tag=f"acc{i}",
                      name=f"acc_l1_{i}") for i in range(IT)]


    def produce_whp(jt):
        pwh = psum.tile([128, NHEADS * NHID], F32, tag="ps")
        nc.tensor.matmul(pwh, xT[:, 128 * jt:128 * (jt + 1)], w_all)
        nc.scalar.copy(out=whp[:, jt, :, 0:NHID],
                       in_=pwh.rearrange("p (h o) -> p h o", h=NHEADS))

    for jt in range(STRIP):       # prime whp strip 0
        produce_whp(jt)
    psp = pet.tile([128, JT, 2 * H_SP], F32, tag="psp")
    for s in range(NSTRIP):
        for jt in range(STRIP * s, STRIP * (s + 1)):
            nc.tensor.matmul(psp[:, jt, :], xT[:, 128 * jt:128 * (jt + 1)], wtilde)
        nc.scalar.activation(out=et_sp[s], in_=psp[:, STRIP * s:STRIP * (s + 1), :],
                             func=Act.Exp)
    for hp, h in enumerate(HEADS):
        if h < H_SP:
            ets, pcol, ccol = et_sp, h, H_SP + h
        else:
            ets, pcol, ccol = et_int, h - H_SP, H_INT + (h - H_SP)
        for s in range(NSTRIP):
            jcs = range(STRIP * s, STRIP * (s + 1))
            if hp == 0 and s + 1 < NSTRIP:   # whp stays one strip ahead
                for jt in range(STRIP * (s + 1), STRIP * (s + 2)):
                    produce_whp(jt)
            t4 = work.tile([128, STRIP, R], MAPDT, tag="t")
            for k, jc in enumerate(jcs):
                nc.vector.tensor_scalar(
                    out=t4[:, k, :], in0=qb[:, h, :],
                    scalar1=ets[s][:, k, pcol:pcol + 1],
                    scalar2=ets[s][:, k, ccol:ccol + 1],
                    op0=Alu.max, op1=Alu.mult)
            m4 = mpool.tile([128, STRIP, R], MAPDT, tag="m")
            if hp % 3 == 1 and hp // 3 < GP_HEADS:
                # unbatched single-tile mask-mults on gpsimd
                for k in range(STRIP):
                    nc.gpsimd.tensor_tensor(
                        m4[:, k, :], t4[:, k, :],
                        adjT_sb[:, STRIP * s + k, :], Alu.mult)
            else:
                nc.vector.tensor_tensor(
                    m4.rearrange("p s i -> p (s i)"), t4.rearrange("p s i -> p (s i)"),
                    adjT_sb[:, STRIP * s:STRIP * (s + 1), :].rearrange("p s i -> p (s i)"),
                    Alu.mult)
            for k, jc in enumerate(jcs):
                for it in range(IT):
                    nc.tensor.matmul(accs[it][:, h, :],
                                     m4[:, k, 128 * it:128 * (it + 1)],
                                     whp[:, jc, h, :],
                                     start=(jc == 0), stop=(jc == JT - 1))

    if nc.dbg_et is not None:
        qbf = big.tile([128, NHEADS, R], F32, tag="qbf")
        nc.scalar.copy(out=qbf, in_=qb)
        nc.sync.dma_start(out=nc.dbg_qb.ap(), in_=qbf)
        accf = big.tile([128, IT, NHEADS * (NHID + 1)], F32, tag="accf")
        for it in range(IT):
            nc.scalar.copy(out=accf[:, it, :],
                           in_=accs[it].rearrange("p h c -> p (h c)"))
        nc.sync.dma_start(out=nc.dbg_acc.ap(), in_=accf)

    # ---------------- h = elu(num/den) -------------------------------------
    hT = big.tile([128, 2, R], MAPDT, tag="hT")
    h_nat = big.tile([128, IT, NHEADS * NHID], MAPDT, tag="h_nat")
    for it in range(IT):
        rec = ext.tile([128, NHEADS], F32, tag="rec")
        nc.vector.reciprocal(out=rec, in_=accs[it][:, :, NHID])
        v = ext.tile([128, NHEADS, NHID], MAPDT, tag="v")
        nc.vector.tensor_tensor(v, accs[it][:, :, 0:NHID],
                                rec.broadcast_to([128, NHEADS, NHID]),
                                Alu.mult)
        e = ext.tile([128, NHEADS * NHID], MAPDT, tag="e")
        nc.scalar.activation(out=e, in_=v.rearrange("p h o -> p (h o)"),
                             func=Act.Exp)
        r = ext.tile([128, NHEADS * NHID], MAPDT, tag="r")
        nc.scalar.activation(out=r, in_=v.rearrange("p h o -> p (h o)"),
                             func=Act.Relu)
        em1 = ext.tile([128, NHEADS * NHID], MAPDT, tag="em1")
        nc.vector.tensor_scalar(out=em1, in0=e, scalar1=-1.0, scalar2=None,
                                op0=Alu.add)
        nc.vector.tensor_tensor(h_nat[:, it, :], em1, r, Alu.min)

    # ---------------- Who, o1/o2 -------------------------------------------
    if MAPDT == F32:
        id_map = ident
    else:
        ident_b = big.tile([128, 128], BF16, tag="ident_b")
        make_identity(nc, ident_b)
        id_map = ident_b
    for fc in range(2):
        ph = psum.tile([128, R], MAPDT, tag="ps")
        for it in range(IT):
            tr(ph[:, 128 * it:128 * (it + 1)],
               h_nat[:, it, 128 * fc:128 * (fc + 1)], id_map)
        nc.scalar.copy(out=hT[:, fc, :], in_=ph)
    wout_m = big.tile([128, 2, NOUT], MAPDT, tag="wout_m")
    nc.scalar.copy(out=wout_m, in_=wout_f)
    pwho = psum.tile([NOUT, R], F32, tag="ps")
    for fc in range(2):
        nc.tensor.matmul(pwho, wout_m[:, fc, :], hT[:, fc, :],
                         start=(fc == 0), stop=(fc == 1))
    whoT = big.tile([NOUT, R], MAPDT, tag="whoT")
    nc.scalar.copy(out=whoT, in_=pwho)
    aout_m = big.tile([NOUT, 2], MAPDT, tag="aout_m")
    nc.scalar.copy(out=aout_m, in_=aout_sb)
    po1 = psum.tile([1, R], F32, tag="ps")
    nc.tensor.matmul(po1, aout_m[:, 0:1], whoT)
    po2s = big.tile([1, R], F32, tag="po2s")
    po2 = psum.tile([1, R], F32, tag="ps")
    nc.tensor.matmul(po2, aout_m[:, 1:2], whoT)
    nc.scalar.copy(out=po2s, in_=po2)
    # Qo row = exp(-0.8 o1), broadcast via K=1 matmul
    qo_row = big.tile([1, R], F32, tag="qo_row")
    nc.scalar.activation(out=qo_row, in_=po1, func=Act.Exp, scale=-0.8)
    qo_m = big.tile([1, R], MAPDT, tag="qo_m")
    nc.scalar.copy(out=qo_m, in_=qo_row)
    qob = big.tile([128, R], MAPDT, tag="qob")
    pqob = psum.tile([128, R], F32, tag="ps")
    nc.tensor.matmul(pqob, ones1, qo_m)
    nc.scalar.copy(out=qob, in_=pqob)

    # ---------------- payload [R, 67] built transposed ---------------------
    # cols 0:64 Who, 64 ones, 65 Po = exp(0.8 o2), 66 Co = exp(0.2 o2)
    payT = big.tile([128, IT, NOUT + 3], MAPDT, tag="payT")
    nc.vector.memset(payT[:, :, NOUT:NOUT + 1], 1.0)
    ccin = dram.tile([R, NOUT + 3], MAPDT, tag="ccin")
    ccout = dram.tile([N, NOUT + 3], MAPDT, tag="ccout")
    for k in range(IT):
        ppt = psum.tile([128, NOUT], MAPDT, tag="ps")
        tr(ppt, whoT[:, 128 * k:128 * (k + 1)], id_map)
        po2t = psum.tile([128, 1], F32, tag="ps")
        tr(po2t, po2s[:, 128 * k:128 * (k + 1)], ident)
        nc.scalar.copy(out=payT[:, k, 0:NOUT], in_=ppt)
        nc.scalar.activation(out=payT[:, k, NOUT + 1:NOUT + 2], in_=po2t,
                             func=Act.Exp, scale=0.8)
        nc.scalar.activation(out=payT[:, k, NOUT + 2:NOUT + 3], in_=po2t,
                             func=Act.Exp, scale=0.2)
    nc.sync.dma_start(out=ccin.rearrange("(k p) c -> p k c", p=128), in_=payT)
    if os.environ.get("KERNEL_SIMCC"):
        for d in range(NCORES):
            nc.sync.dma_start(out=ccout[R * d:R * (d + 1), :], in_=ccin)
    else:
        nc.gpsimd.collective_compute(
            "AllGather", mybir.AluOpType.bypass,
            replica_groups=[list(range(NCORES))],
            ins=[ccin.opt()], outs=[ccout.opt()])
    whop = big.tile([128, JT, NOUT + 3], MAPDT, tag="whop")
    nc.sync.dma_start(out=whop, in_=ccout.rearrange("(t p) c -> p t c", p=128))
    pco = big.tile([128, JT, 2], F32, tag="pco")
    nc.scalar.copy(out=pco, in_=whop[:, :, NOUT + 1:NOUT + 3])

    # ---------------- output attention -------------------------------------
    acc2 = [pacc.tile([128, NOUT + 1], F32, tag=f"acc{i}",
                      name=f"acc_l2_{i}") for i in range(IT)]
    for s in range(NSTRIP):
        jcs = range(STRIP * s, STRIP * (s + 1))
        t4 = work.tile([128, STRIP, R], MAPDT, tag="t")
        for k, jc in enumerate(jcs):
            nc.vector.tensor_scalar(
                out=t4[:, k, :], in0=qob,
                scalar1=pco[:, jc, 0:1], scalar2=pco[:, jc, 1:2],
                op0=Alu.max, op1=Alu.mult)
        m4 = mpool.tile([128, STRIP, R], MAPDT, tag="m")
        if os.environ.get("KERNEL_UNBATCH_TT"):
            for k, jc in enumerate(jcs):
                nc.vector.tensor_tensor(
                    m4[:, k, :], t4[:, k, :],
                    adjT_sb[:, jc, :], Alu.mult)
        else:
            for k in range(GP_OUT):
                nc.gpsimd.tensor_tensor(
                    m4[:, k, :], t4[:, k, :],
                    adjT_sb[:, STRIP * s + k, :], Alu.mult)
            if GP_OUT < STRIP:
                nc.vector.tensor_tensor(
                    m4[:, GP_OUT:, :].rearrange("p s i -> p (s i)"),
                    t4[:, GP_OUT:, :].rearrange("p s i -> p (s i)"),
                    adjT_sb[:, STRIP * s + GP_OUT:STRIP * (s + 1), :]
                        .rearrange("p s i -> p (s i)"),
                    Alu.mult)
        for k, jc in enumerate(jcs):
            for it in range(IT):
                nc.tensor.matmul(acc2[it],
                                 m4[:, k, 128 * it:128 * (it + 1)],
                                 whop[:, jc, 0:NOUT + 1],
                                 start=(jc == 0), stop=(jc == JT - 1))

    # ---------------- out = tanh(num/den) ----------------------------------
    out_sb = big.tile([128, IT, NOUT], F32, tag="out_sb")
    for it in range(IT):
        rec2 = ext.tile([128, 1], F32, tag="rec2")
        nc.vector.reciprocal(out=rec2, in_=acc2[it][:, NOUT:NOUT + 1])
        nc.scalar.activation(out=out_sb[:, it, :], in_=acc2[it][:, 0:NOUT],
                             func=Act.Tanh, scale=rec2)
    nc.sync.dma_start(out=d_out.ap().rearrange("(k p) c -> p k c", p=128),
                      in_=out_sb)


_NC_CACHE = None


def _get_nc():
    global _NC_CACHE
    if _NC_CACHE is None:
        _NC_CACHE = _build_program()
    return _NC_CACHE


def _make_in_maps(inputs):
    x = np.asarray(inputs["x"], np.float32)
    adj = np.asarray(inputs["adj"], np.float32)
    ie = np.asarray(inputs["intent_embeds"], np.float32)
    xT_full = np.ascontiguousarray(x.T)
    ieT_full = np.ascontiguousarray(ie.T)
    in_maps = []
    for d in range(NCORES):
        sl = slice(d * R, (d + 1) * R)
        in_maps.append({
            "xT": xT_full, "ieT": ieT_full,
            "adjT": np.ascontiguousarray(adj[sl, :].T).astype(NPMAP),
            "xoT": np.ascontiguousarray(x[sl].T),
            "ioT": np.ascontiguousarray(ie[sl].T),
            "wsp": np.asarray(inputs["W_sp"], np.float32),
            "asp": np.asarray(inputs["a_sp"], np.float32),
            "wint": np.asarray(inputs["W_int"], np.float32),
            "aint": np.asarray(inputs["a_int"], np.float32),
            "wout": np.asarray(inputs["W_out"], np.float32),
            "aout": np.asarray(inputs["a_out"], np.float32),
        })
    return in_maps


def kernel(x, adj, intent_embeds, W_sp, a_sp, W_int, a_int, W_out, a_out):
    nc = _get_nc()
    in_maps = _make_in_maps(dict(
        x=x, adj=adj, intent_embeds=intent_embeds, W_sp=W_sp, a_sp=a_sp,
        W_int=W_int, a_int=a_int, W_out=W_out, a_out=a_out))
    res = run_bass_kernel_spmd(nc, in_maps, list(range(NCORES)))
    return np.concatenate([res.results[d]["out"] for d in range(NCORES)], axis=0)
